# revision 1
# baseline (speedup 1.0000x reference)
"""MoE (top-2 of 8 experts) Trainium2 kernel.

Strategy: expert-parallel across the 8 NeuronCores. The router
(softmax + top-2 over [T, 8] logits) is metadata computed on host to
build the dispatch; core e receives only the tokens routed to expert e
(gathered, transposed, zero-padded to a common capacity C) plus that
expert's weights, pre-transposed so the device does no transposes:

  core e inputs:  xT  [H, C]   = x[idx_e].T        (padded)
                  w1T [H, I]   = w1[e].T
                  w2T [I, H]   = w2[e].T
                  gates [1, C]  renormalized top-2 weight per token
  core e output:  yT  [H, C]  = (gate * (silu(x_e @ w1[e].T) @ w2[e].T)).T

On device (per core, fp32 storage, float32r matmuls, only the exact
even-rounded token count is computed — no padding columns):
  stage 1: hT[i_tile, c_chunk] = silu(w1T.T @ xT)    (I on partitions)
  stage 2: yT[h_tile, c_chunk] = w2T.T @ hT, times the per-token gate
           (w2 stationary, hT moving: the stream covers the ragged token
           dim; gate is broadcast to all partitions by a 0-stride DMA)

The host transposes and scatter-adds the two expert contributions per
token.
"""

import numpy as np

import concourse.mybir as mybir
from concourse import bacc
from concourse.tile import TileContext
from concourse.bass_utils import run_bass_kernel_spmd

T, H, I, E = 4096, 1024, 1408, 8
TOPK = 2
P = 128
CHUNK = 512
N_CORES = 8
F32 = mybir.dt.float32
F32R = mybir.dt.float32r
AF = mybir.ActivationFunctionType

# most recently built device program (for test harnesses / cost-model timing)
LAST_NC = None


def _chunk_sizes(C):
    """Split C into ceil(C/512) chunks, multiples of 128, as even as
    possible. Balanced chunks keep every stage-1 matmul's moving dim >=256
    (the fp32r full-rate threshold) instead of a slow ragged tail."""
    n = -(-C // CHUNK)
    base = (C // n) // P * P
    rem = (C - n * base) // P
    return [base + P if j < rem else base for j in range(n)]


def _count_chunks(count):
    """Chunk an exact token count (no alignment needed: both stages
    stream the token dim). Front chunks are full 512 so stage-1 groups outlast
    the w1 tile arrival period (no DMA-pacing stalls during the weight
    stream); the tail is split to keep every chunk >=256 (the fp32r
    full-rate threshold) whenever count allows."""
    full, rem = divmod(count, CHUNK)
    if rem == 0:
        return [CHUNK] * full
    if rem >= 256 or full == 0:
        return [CHUNK] * full + [rem]
    # rem < 256: borrow from one full chunk so both tail chunks stay >=256
    return [CHUNK] * (full - 1) + [256 + rem, 256]


def build_moe_expert_kernel(count, h=H, i_dim=I):
    """One-expert MLP over `count` gathered tokens (any positive int —
    DRAM buffers are padded to a 128 multiple, but only `count` columns
    are computed). h, i_dim overridable for small-scale simulation tests;
    both must be multiples of 128. count must be even (fp32r matmuls
    reject odd free/partition sizes)."""
    C = -(-count // P) * P  # DRAM/layout capacity
    assert count % 2 == 0 and h % P == 0 and i_dim % P == 0
    HK = h // P
    IT = i_dim // P

    nc = bacc.Bacc("TRN2", target_bir_lowering=False, debug=False, num_devices=N_CORES)
    # Matmul inputs are stored as float32r (same 32-bit layout; the PE
    # rounds to its reduced internal precision). Typing the whole producer
    # chain as f32r satisfies the BIR verifier's rounding check.
    xT = nc.dram_tensor("xT", [h, C], F32R, kind="ExternalInput").ap()
    w1T = nc.dram_tensor("w1T", [h, i_dim], F32R, kind="ExternalInput").ap()
    w2T = nc.dram_tensor("w2T", [i_dim, h], F32R, kind="ExternalInput").ap()
    gates = nc.dram_tensor("gates", [1, C], F32, kind="ExternalInput").ap()
    # host-packed first-group operands: per partition p (= h row p),
    # [w1T[p, 0:128] | xT[p, 0:cs0]] — one DMA arms the first matmul
    cs0_pre = _count_chunks(count)[0]
    prelude = nc.dram_tensor("prelude", [P, P + cs0_pre], F32R, kind="ExternalInput").ap()
    # output is yT [h, C]: stage 2 streams over the ragged token dim, so
    # tokens land on the free axis (the host transposes back)
    yT = nc.dram_tensor("yT", [h, C], F32, kind="ExternalOutput").ap()

    xT_v = xT.rearrange("(ho p) c -> p ho c", p=P)  # [128, HK, C]
    w1T_v = w1T.rearrange("(ho p) i -> p ho i", p=P)  # [128, HK, I]
    w2T_v = w2T.rearrange("(io p) h -> p io h", p=P)  # [128, IT, H]
    yT_v = yT.rearrange("(ho p) c -> ho p c", p=P)  # [HK, 128, C]

    h_chunks = _chunk_sizes(h)  # h-chunks for stage 2 output
    c_chunks = _count_chunks(count)
    max_cs = max(c_chunks)
    c_starts = [sum(c_chunks[:j]) for j in range(len(c_chunks))]
    # per-partition SBUF bytes: weights + broadcast gates + h/sg bufs; give
    # the x and y pools extra bufs only while the 192 KB budget holds
    base = 4 * (HK * i_dim + IT * h + C + 2 * IT * max_cs + 2 * CHUNK)
    x_bufs = 3 if base + 3 * 4 * HK * max_cs + 2 * 4 * CHUNK < 190 * 1024 else 2
    fixed = base + x_bufs * 4 * HK * max_cs
    y_bufs = 4 if fixed + 4 * 4 * CHUNK < 190 * 1024 else 2
    with TileContext(nc) as tc:
        with (
            tc.tile_pool(name="wpool", bufs=1) as wpool,
            tc.tile_pool(name="xpool", bufs=x_bufs) as xpool,
            tc.tile_pool(name="hpool", bufs=2) as hpool,
            tc.tile_pool(name="ypool", bufs=y_bufs) as ypool,
            tc.tile_pool(name="sgpool", bufs=2) as sgpool,
            tc.tile_pool(name="ps1", bufs=4, space="PSUM") as ps1pool,
            tc.tile_pool(name="ps2", bufs=4, space="PSUM") as ps2pool,
        ):
            # per-token gate replicated to all partitions: one DMA reading
            # the same DRAM row 128x (0-stride partition source)
            gb = wpool.tile([P, C], F32)
            w1s = wpool.tile([P, HK, i_dim], F32R)
            w2s = wpool.tile([P, IT, h], F32R)
            xs_tiles = {}

            def load_x(ci, split=True):
                # per-hk DMAs deliver the chunk incrementally so stage-1
                # groups can start before the whole chunk lands
                xs = xpool.tile([P, HK, max_cs], F32R, tag="xs", name=f"xs{ci}")
                cs, c0 = c_chunks[ci], c_starts[ci]
                if split:
                    for hk in range(HK):
                        nc.sync.dma_start(xs[:, hk, :cs], xT_v[:, hk, c0 : c0 + cs])
                else:
                    nc.sync.dma_start(xs[:, :, :cs], xT_v[:, :, c0 : c0 + cs])
                xs_tiles[ci] = xs

            def load_w1(it):
                nc.sync.dma_start(
                    w1s[:, :, it * P : (it + 1) * P],
                    w1T_v[:, :, it * P : (it + 1) * P],
                )

            # DMA issue order = consumption order. Interleave chunk-0 x
            # slices with the leading w1 i-tiles so the first stage-1
            # accumulation group starts after ~0.7 MB instead of ~6 MB;
            # then the rest of w1, the remaining x chunks, then w2 (per
            # h-half, consumed by stage 2).
            xs0 = xpool.tile([P, HK, max_cs], F32R, tag="xs", name="xs0")
            cs0 = c_chunks[0]
            pre = wpool.tile([P, P + cs0], F32R)
            nc.sync.dma_start(pre[:], prelude[:])
            # w1 it0's hk0 slice lives in the prelude; load only hk1..
            nc.sync.dma_start(w1s[:, 1:, 0:P], w1T_v[:, 1:, 0:P])
            for hk in range(1, HK):
                nc.sync.dma_start(xs0[:, hk, :cs0], xT_v[:, hk, 0:cs0])
                if hk == min(2, HK - 1) and IT > 1:
                    load_w1(1)
            xs_tiles[0] = xs0
            for it in range(2, IT):
                load_w1(it)
            # w2 per h-half per i-tile: stage 2 consumes one h-chunk across
            # i-tiles in order, so fine-grained delivery unblocks each
            # accumulation group as early as possible
            h_starts = [sum(h_chunks[:j]) for j in range(len(h_chunks))]
            # only as many x chunks up front as there are pool slots — a
            # queued DMA waiting on a busy slot would head-of-line block
            # the w2 stream behind it; later chunks prefetch inside stage 1
            for ci in range(1, min(x_bufs, len(c_chunks))):
                load_x(ci)
            # broadcast-gate load sits after the stage-1 streams (it is
            # only needed when the first stage-2 group's psum is evacuated)
            nc.sync.dma_start(gb[:], gates[0].partition_broadcast(P))
            for h0, hcs in zip(h_starts, h_chunks):
                for it in range(IT):
                    nc.sync.dma_start(
                        w2s[:, it, h0 : h0 + hcs], w2T_v[:, it, h0 : h0 + hcs]
                    )

            hs_tiles = {}

            def stage1(ci):
                nxt = ci + 1
                if nxt < len(c_chunks) and nxt not in xs_tiles:
                    load_x(nxt)
                cs = c_chunks[ci]
                xs = xs_tiles[ci]
                # hT = silu(w1T.T @ xT)  -> [I, cs], I on partitions
                hs = hpool.tile([P, IT, max_cs], F32R, tag="hs", name=f"hs{ci}")
                for it in range(IT):
                    ps1 = ps1pool.tile([P, CHUNK], F32, tag="ps1")
                    for hk in range(HK):
                        # (it0, hk0) weights and chunk-0's hk0 x-slice live
                        # in the prelude tile (w1s[:, 0, 0:P] is never DMA'd)
                        if hk == 0 and it == 0:
                            lhsT = pre[:, 0:P]
                        else:
                            lhsT = w1s[:, hk, it * P : (it + 1) * P]
                        if ci == 0 and hk == 0:
                            rhs = pre[:, P : P + cs]
                        else:
                            rhs = xs[:, hk, :cs]
                        nc.tensor.matmul(
                            ps1[:, :cs],
                            lhsT,
                            rhs,
                            start=(hk == 0),
                            stop=(hk == HK - 1),
                        )
                    # silu(z) = z * sigmoid(z); CoreSim has no Silu table,
                    # so build it from Sigmoid (ACT) + multiply (DVE)
                    sg = sgpool.tile([P, CHUNK], F32, tag="sg")
                    nc.scalar.activation(sg[:, :cs], ps1[:, :cs], AF.Sigmoid)
                    nc.vector.tensor_mul(
                        out=hs[:, it, :cs], in0=ps1[:, :cs], in1=sg[:, :cs]
                    )
                hs_tiles[ci] = hs

            def stage2(ci):
                # yT = (w2T.T @ hT) * gate -> [H, cs], h on partitions.
                # w2 is the stationary operand and hT the moving one, so the
                # stream covers exactly the ragged token count — no padded
                # columns and no partial-partition tiles.
                cs, c0 = c_chunks[ci], c_starts[ci]
                hs = hs_tiles.pop(ci)
                for ht in range(HK):
                    ps2 = ps2pool.tile([P, CHUNK], F32, tag="ps2")
                    for it in range(IT):
                        nc.tensor.matmul(
                            ps2[:, :cs],
                            w2s[:, it, ht * P : (ht + 1) * P],
                            hs[:, it, :cs],
                            start=(it == 0),
                            stop=(it == IT - 1),
                        )
                    ys = ypool.tile([P, CHUNK], F32, tag="ys")
                    nc.vector.tensor_mul(
                        out=ys[:, :cs], in0=ps2[:, :cs], in1=gb[:, c0 : c0 + cs]
                    )
                    nc.sync.dma_start(yT_v[ht][:, c0 : c0 + cs], ys[:, :cs])

            # software pipeline: run stage 1 a chunk ahead so the PE has
            # stage-1 work for chunk i+1 while w2 is still streaming in
            stage1(0)
            for ci in range(1, len(c_chunks)):
                stage1(ci)
                stage2(ci - 1)
            stage2(len(c_chunks) - 1)
    nc.compile()
    global LAST_NC
    LAST_NC = nc
    return nc


def route(router_logits):
    """Host-side router: softmax -> top-2 -> renormalize.

    Returns (top2_idx [T,2] int64, top2_gate [T,2] float32)."""
    logits = np.asarray(router_logits, dtype=np.float32)
    m = logits.max(axis=-1, keepdims=True)
    ex = np.exp(logits - m)
    probs = ex / ex.sum(axis=-1, keepdims=True)
    order = np.argsort(-probs, axis=-1, kind="stable")[:, :TOPK]
    rows = np.arange(logits.shape[0])[:, None]
    topk_p = probs[rows, order]
    topk_p = topk_p / topk_p.sum(axis=-1, keepdims=True)
    return order, topk_p.astype(np.float32)


def kernel(x, router_logits, w1, w2):
    x = np.ascontiguousarray(np.asarray(x, dtype=np.float32))
    w1 = np.asarray(w1, dtype=np.float32)
    w2 = np.asarray(w2, dtype=np.float32)
    t = x.shape[0]

    top2_idx, top2_gate = route(router_logits)

    expert_tokens = []
    expert_gates = []
    for e in range(E):
        sel = np.nonzero(top2_idx == e)
        expert_tokens.append(sel[0])
        expert_gates.append(top2_gate[sel[0], sel[1]])
    counts = [len(ix) for ix in expert_tokens]
    # fp32r matmuls require even free/partition sizes (2-element PSUM
    # interleave), so round the computed token count up to even
    count = max(2, max(counts) + max(counts) % 2)
    C = -(-count // P) * P  # buffer capacity (128-aligned)

    nc = build_moe_expert_kernel(count)
    kernel_cs0 = _count_chunks(count)[0]

    in_maps = []
    for e in range(E):
        cnt = counts[e]
        xT_e = np.zeros((H, C), dtype=np.float32)
        xT_e[:, :cnt] = x[expert_tokens[e]].T
        g = np.zeros((1, C), dtype=np.float32)
        g[0, :cnt] = expert_gates[e]
        w1T_e = np.ascontiguousarray(w1[e].T)
        cs0 = kernel_cs0
        in_maps.append(
            {
                "xT": xT_e,
                "w1T": w1T_e,
                "w2T": np.ascontiguousarray(w2[e].T),
                "gates": g,
                "prelude": np.ascontiguousarray(
                    np.concatenate([w1T_e[:P, :P], xT_e[:P, :cs0]], axis=1)
                ),
            }
        )

    res = run_bass_kernel_spmd(nc, in_maps, core_ids=list(range(N_CORES)))
    if not all(np.isfinite(r["yT"]).all() for r in res.results):
        # one retry in case of a transient device fault
        res = run_bass_kernel_spmd(nc, in_maps, core_ids=list(range(N_CORES)))

    out = np.zeros((t, H), dtype=np.float32)
    for e in range(E):
        cnt = counts[e]
        out[expert_tokens[e]] += res.results[e]["yT"][:, :cnt].T
    return out



# revision 2
# speedup vs baseline: 1.1883x; 1.1883x over previous
"""MoE (top-2 of 8 experts) Trainium2 kernel — fp8 DoubleRow edition.

Strategy: expert-parallel across the 8 NeuronCores (host router builds the
dispatch; core e gets the tokens routed to expert e). The expert MLP runs
as 3-term residual-split fp8 matmuls in DoubleRow perf mode (contraction
256 per pass, 0.5 PE cycles per output column — 4x the bf16 MAC rate):

  A ≈ Ah + Al  (Ah = e4m3(s*A), Al = e4m3(s*A - Ah): the RN residual is
  exactly representable in e4m3, so Ah+Al carries ~9 significant bits)

  A@B ≈ Ah@Bh + Ah@Bl + Al@Bh   (lo*lo term dropped, ~1e-3 rel)

Per-core pipeline (count tokens, chunks of <=512 on the free axis):
  stage 1: ps1 = 16*z = sum of 3 DoubleRow terms (w1 planes x x planes)
           sg = sigmoid(ps1/16)         [ACT]
           t  = ps1*sg = 16*silu(z)     [DVE, bf16]
           Hh = e4m3(t)                 [ACT copy]
           Hl = e4m3(t - Hh)            [DVE fused (t*1)-Hh]
  stage 2: ps2 = 512*y = 3 DoubleRow terms (w2 planes x H planes),
           I padded to 1536 (12 i-tiles) with zero weights/h
           yv = bf16(ps2)               [ACT copy] -> DMA out
  host:    out[tok] += gate/512 * yv.T  (gate multiply + dequant on host)

All DRAM operands are host-tiled so every DMA lands with >=512B
contiguous runs and block granularity matching PE consumption order.
"""

import numpy as np
import ml_dtypes

import concourse.mybir as mybir
from concourse import bacc
from concourse.tile import TileContext
from concourse.bass_utils import run_bass_kernel_spmd

T, H, I, E = 4096, 1024, 1408, 8
TOPK = 2
P = 128
HK = H // P  # 8 h-tiles (stage-1 contraction, stage-2 output)
IT = I // P  # 11 i-tiles (stage-1 output)
IT2 = IT + 1  # stage-2 contraction padded to 12 tiles (6 DoubleRow pairs)
CHUNK = 512
N_CORES = 8

F32 = mybir.dt.float32
BF16 = mybir.dt.bfloat16
F8 = mybir.dt.float8e4
AF = mybir.ActivationFunctionType
DR = mybir.MatmulPerfMode.DoubleRow
E4NP = ml_dtypes.float8_e4m3

SX = 2.0  # x plane scale
SW1 = 8.0  # w1 plane scale ->  ps1 = 16*z
SW2 = 32.0  # w2 plane scale ->  ps2 = 512*y
PS1_SCALE = SX * SW1
PS2_SCALE = PS1_SCALE * SW2

# most recently built device program (for test harnesses / cost-model timing)
LAST_NC = None


def _chunks(count):
    out = []
    r = count
    while r > 0:
        c = min(CHUNK, r)
        out.append(c)
        r -= c
    return out


def build_moe_expert_kernel(count):
    """One-expert MLP over `count` gathered tokens (count even)."""
    C = count
    assert count % 2 == 0
    c_chunks = _chunks(count)
    c_starts = [sum(c_chunks[:j]) for j in range(len(c_chunks))]
    n_chunks = len(c_chunks)

    nc = bacc.Bacc("TRN2", target_bir_lowering=False, debug=False, num_devices=N_CORES)
    xh = nc.dram_tensor("xh", [P, HK, C], F8, kind="ExternalInput").ap()
    xl = nc.dram_tensor("xl", [P, HK, C], F8, kind="ExternalInput").ap()
    w1h = nc.dram_tensor("w1h", [IT, P, HK, P], F8, kind="ExternalInput").ap()
    w1l = nc.dram_tensor("w1l", [IT, P, HK, P], F8, kind="ExternalInput").ap()
    w2h = nc.dram_tensor("w2h", [HK, P, IT2, P], F8, kind="ExternalInput").ap()
    w2l = nc.dram_tensor("w2l", [HK, P, IT2, P], F8, kind="ExternalInput").ap()
    y = nc.dram_tensor("y", [P, HK, C], BF16, kind="ExternalOutput").ap()

    with TileContext(nc) as tc:
        with (
            tc.tile_pool(name="wpool", bufs=1) as wpool,
            tc.tile_pool(name="xpool", bufs=min(3, n_chunks)) as xpool,
            tc.tile_pool(name="hpool", bufs=min(2, n_chunks)) as hpool,
            tc.tile_pool(name="spool", bufs=2) as spool,
            tc.tile_pool(name="ypool", bufs=min(2, n_chunks)) as ypool,
            tc.tile_pool(name="ps1", bufs=4, space="PSUM") as ps1pool,
            tc.tile_pool(name="ps2", bufs=4, space="PSUM") as ps2pool,
        ):
            w1hs = wpool.tile([P, IT, HK, P], F8)
            w1ls = wpool.tile([P, IT, HK, P], F8)
            w2hs = wpool.tile([P, HK, IT2, P], F8)
            w2ls = wpool.tile([P, HK, IT2, P], F8)

            xh_t, xl_t = {}, {}

            def load_x(ci, split_hi=False):
                cs, c0 = c_chunks[ci], c_starts[ci]
                th = xpool.tile([P, HK, CHUNK], F8, tag="xh", name=f"xh{ci}")
                tl = xpool.tile([P, HK, CHUNK], F8, tag="xl", name=f"xl{ci}")
                if split_hi:
                    # hk-pair slabs so the first stage-1 group can start
                    # after ~0.13 MB instead of the whole plane
                    for k in range(HK // 2):
                        nc.sync.dma_start(
                            th[:, 2 * k : 2 * k + 2, :cs],
                            xh[:, 2 * k : 2 * k + 2, c0 : c0 + cs],
                        )
                else:
                    nc.sync.dma_start(th[:, :, :cs], xh[:, :, c0 : c0 + cs])
                nc.sync.dma_start(tl[:, :, :cs], xl[:, :, c0 : c0 + cs])
                xh_t[ci], xl_t[ci] = th, tl

            # DMA issue order = consumption order: w1h it0, the first x
            # chunk, then the full w1 stream, remaining x chunks, then w2.
            nc.sync.dma_start(w1hs[:, 0, :, :], w1h[0])
            load_x(0, split_hi=True)
            nc.sync.dma_start(w1ls[:, 0, :, :], w1l[0])
            for it in range(1, IT):
                nc.sync.dma_start(w1hs[:, it, :, :], w1h[it])
                nc.sync.dma_start(w1ls[:, it, :, :], w1l[it])
            for ci in range(1, n_chunks):
                load_x(ci)
            for ht in range(HK):
                nc.sync.dma_start(w2hs[:, ht, :, :], w2h[ht])
                nc.sync.dma_start(w2ls[:, ht, :, :], w2l[ht])

            hh_t, hl_t = {}, {}

            def stage1(ci):
                cs = c_chunks[ci]
                xht, xlt = xh_t[ci], xl_t[ci]
                hh = hpool.tile([P, IT2, CHUNK], F8, tag="hh", name=f"hh{ci}")
                hl = hpool.tile([P, IT2, CHUNK], F8, tag="hl", name=f"hl{ci}")
                # stage-2 reads the zero-pad i-tile via its DoubleRow pair
                nc.vector.memset(hh[:, IT, :cs], 0.0)
                nc.vector.memset(hl[:, IT, :cs], 0.0)
                for it in range(IT):
                    ps1 = ps1pool.tile([P, CHUNK], F32, tag="ps1")
                    terms = ((w1hs, xht), (w1hs, xlt), (w1ls, xht))
                    for ti, (ws, xs) in enumerate(terms):
                        for k in range(HK // 2):
                            nc.tensor.matmul(
                                ps1[:, :cs],
                                ws[:, it, 2 * k : 2 * k + 2, :],
                                xs[:, 2 * k : 2 * k + 2, :cs],
                                start=(ti == 0 and k == 0),
                                stop=(ti == 2 and k == HK // 2 - 1),
                                perf_mode=DR,
                            )
                    sg = spool.tile([P, CHUNK], F32, tag="sg")
                    t = spool.tile([P, CHUNK], BF16, tag="t")
                    nc.scalar.activation(
                        sg[:, :cs], ps1[:, :cs], AF.Sigmoid, scale=1.0 / PS1_SCALE
                    )
                    nc.vector.tensor_mul(out=t[:, :cs], in0=ps1[:, :cs], in1=sg[:, :cs])
                    nc.scalar.activation(hh[:, it, :cs], t[:, :cs], AF.Copy)
                    nc.vector.scalar_tensor_tensor(
                        out=hl[:, it, :cs],
                        in0=t[:, :cs],
                        scalar=1.0,
                        in1=hh[:, it, :cs],
                        op0=mybir.AluOpType.mult,
                        op1=mybir.AluOpType.subtract,
                    )
                hh_t[ci], hl_t[ci] = hh, hl

            def stage2(ci):
                cs, c0 = c_chunks[ci], c_starts[ci]
                hh, hl = hh_t.pop(ci), hl_t.pop(ci)
                yv = ypool.tile([P, HK, CHUNK], BF16, tag="yv", name=f"yv{ci}")
                for ht in range(HK):
                    ps2 = ps2pool.tile([P, CHUNK], F32, tag="ps2")
                    terms = ((w2hs, hh), (w2hs, hl), (w2ls, hh))
                    for ti, (ws, hs) in enumerate(terms):
                        for k in range(IT2 // 2):
                            nc.tensor.matmul(
                                ps2[:, :cs],
                                ws[:, ht, 2 * k : 2 * k + 2, :],
                                hs[:, 2 * k : 2 * k + 2, :cs],
                                start=(ti == 0 and k == 0),
                                stop=(ti == 2 and k == IT2 // 2 - 1),
                                perf_mode=DR,
                            )
                    nc.scalar.activation(yv[:, ht, :cs], ps2[:, :cs], AF.Copy)
                nc.sync.dma_start(y[:, :, c0 : c0 + cs], yv[:, :, :cs])

            # software pipeline: stage 1 runs a chunk ahead so stage-2 has
            # its H planes ready and the w2 stream time to land
            stage1(0)
            for ci in range(1, n_chunks):
                stage1(ci)
                stage2(ci - 1)
            stage2(n_chunks - 1)
    nc.compile()
    global LAST_NC
    LAST_NC = nc
    return nc


def route(router_logits):
    """Host-side router: softmax -> top-2 -> renormalize."""
    logits = np.asarray(router_logits, dtype=np.float32)
    m = logits.max(axis=-1, keepdims=True)
    ex = np.exp(logits - m)
    probs = ex / ex.sum(axis=-1, keepdims=True)
    order = np.argsort(-probs, axis=-1, kind="stable")[:, :TOPK]
    rows = np.arange(logits.shape[0])[:, None]
    topk_p = probs[rows, order]
    topk_p = topk_p / topk_p.sum(axis=-1, keepdims=True)
    return order, topk_p.astype(np.float32)


def _split_e4(a):
    """Residual split: a ~= hi + lo, both e4m3 (RN residual is exact)."""
    hi = np.asarray(a, np.float32).astype(E4NP)
    lo = (a - hi.astype(np.float32)).astype(E4NP)
    return hi, lo


def kernel(x, router_logits, w1, w2):
    x = np.ascontiguousarray(np.asarray(x, dtype=np.float32))
    w1 = np.asarray(w1, dtype=np.float32)
    w2 = np.asarray(w2, dtype=np.float32)
    t = x.shape[0]

    top2_idx, top2_gate = route(router_logits)

    expert_tokens = []
    expert_gates = []
    for e in range(E):
        sel = np.nonzero(top2_idx == e)
        expert_tokens.append(sel[0])
        expert_gates.append(top2_gate[sel[0], sel[1]])
    counts = [len(ix) for ix in expert_tokens]
    count = max(2, max(counts) + max(counts) % 2)
    C = count

    nc = build_moe_expert_kernel(count)

    in_maps = []
    for e in range(E):
        cnt = counts[e]
        xT = np.zeros((H, C), dtype=np.float32)
        xT[:, :cnt] = x[expert_tokens[e]].T
        Xh, Xl = _split_e4(SX * xT)
        # [H, C] -> [P, HK, C]
        xh_a = np.ascontiguousarray(Xh.reshape(HK, P, C).transpose(1, 0, 2))
        xl_a = np.ascontiguousarray(Xl.reshape(HK, P, C).transpose(1, 0, 2))

        w1T = w1[e].T  # [H, I]
        W1h, W1l = _split_e4(SW1 * w1T)
        # [H, I] -> [IT, P, HK, P]
        w1h_a = np.ascontiguousarray(
            W1h.reshape(HK, P, IT, P).transpose(2, 1, 0, 3)
        )
        w1l_a = np.ascontiguousarray(
            W1l.reshape(HK, P, IT, P).transpose(2, 1, 0, 3)
        )

        w2T = np.zeros((IT2 * P, H), dtype=np.float32)  # [I padded, H]
        w2T[:I] = w2[e].T
        W2h, W2l = _split_e4(SW2 * w2T)
        # [Ipad, H] -> [HK, P, IT2, P]
        w2h_a = np.ascontiguousarray(
            W2h.reshape(IT2, P, HK, P).transpose(2, 1, 0, 3)
        )
        w2l_a = np.ascontiguousarray(
            W2l.reshape(IT2, P, HK, P).transpose(2, 1, 0, 3)
        )

        in_maps.append(
            {
                "xh": xh_a,
                "xl": xl_a,
                "w1h": w1h_a,
                "w1l": w1l_a,
                "w2h": w2h_a,
                "w2l": w2l_a,
            }
        )

    res = run_bass_kernel_spmd(nc, in_maps, core_ids=list(range(N_CORES)))
    ys = [np.asarray(r["y"], dtype=np.float32) for r in res.results]
    if not all(np.isfinite(yy).all() for yy in ys):
        # one retry in case of a transient device fault
        res = run_bass_kernel_spmd(nc, in_maps, core_ids=list(range(N_CORES)))
        ys = [np.asarray(r["y"], dtype=np.float32) for r in res.results]

    out = np.zeros((t, H), dtype=np.float32)
    for e in range(E):
        cnt = counts[e]
        # y dram [P, HK, C] -> [C, H]
        y_e = ys[e].transpose(2, 1, 0).reshape(C, H)
        g = expert_gates[e][:, None] * (1.0 / PS2_SCALE)
        out[expert_tokens[e]] += g * y_e[:cnt]
    return out
